# revision 1
# baseline (speedup 1.0000x reference)
"""CIF (Continuous Integrate-and-Fire) forced-alignment kernel for 8 TRN2 NeuronCores.

Contract: kernel(**inputs) takes the FULL inputs from setup_inputs() and returns
the FULL (32, 8, 768) f32 output. Batch is sharded 4 seqs/core (pure data
parallel); each core runs an identical Bass/Tile program.

Math: Conv1d(D,D,5,SAME) + Linear(D,1) collapse into an effective (5,D) filter
(both are linear); sigmoid -> mask -> normalize to sum L -> cumsum -> bucket
overlap weights w (B,L,T) -> out = w @ x.

Device layout: t = p*J + j (p = partition, J = T/128). Projections via PE
transpose-mode matmuls (bf16), logits via padded shifted adds, cumsum via
free-dim prefix scan + strict-triangular matmul for cross-partition offsets,
einsum via PE matmuls with w stationary.
"""
import sys

if "/opt/trn_rl_repo" not in sys.path:
    sys.path.insert(0, "/opt/trn_rl_repo")

import numpy as np
import ml_dtypes

P = 128
BSZ, T_FULL, D_FULL = 32, 2048, 768
L_OUT = 8
N_CORES = 8
S_PER_CORE = BSZ // N_CORES


def fold_weights(conv_w, conv_b, lin_w, lin_b):
    Weff = np.einsum("o,oik->ki", lin_w[0].astype(np.float64),
                     conv_w.astype(np.float64)).astype(np.float32)  # (5, D)
    beff = float(lin_w[0].astype(np.float64) @ conv_b.astype(np.float64) + lin_b[0])
    return Weff, beff


def make_consts(S, T, D, L, wT, mb):
    """Pack all constants into one f32 blob and one bf16 blob (2 DMAs)."""
    J = T // P
    CH = D // P
    bf = ml_dtypes.bfloat16
    u128x = np.triu(np.ones((P, P), np.float32), 1)         # [p,m]=1 iff p<m
    onescol = np.ones((P, 1), np.float32)
    l = np.arange(L, dtype=np.float32)
    eglo = np.broadcast_to(l[None, :, None], (P, L, J)).astype(np.float32)
    eghi = eglo + 1.0
    onesrow_slot = np.zeros((P, P), np.float32)
    onesrow_slot[0, :] = np.float32(L)
    cf32 = np.concatenate([
        u128x, onescol, eglo.reshape(P, L * J), eghi.reshape(P, L * J),
        mb.reshape(P, -1), onesrow_slot], axis=1)
    ident = np.eye(P, dtype=bf)
    i5_slot = np.zeros((P, 5), bf)
    i5_slot[0:5, 0:5] = np.eye(5, dtype=bf)
    sup = np.zeros((P, P), bf)                              # out[m]=in[m+1]
    sup[np.arange(1, P), np.arange(P - 1)] = 1
    sdn = np.zeros((P, P), bf)                              # out[m]=in[m-1]
    sdn[np.arange(P - 1), np.arange(1, P)] = 1
    cbf16 = np.concatenate([ident, i5_slot, sup, sdn,
                            wT.reshape(P, CH * 5)], axis=1)
    return dict(cf32=np.ascontiguousarray(cf32),
                cbf16=np.ascontiguousarray(cbf16))


def make_core_inputs(x_shard, lens_shard, Weff, beff, S, T, D, L, jlens=None):
    J = T // P
    CH = D // P
    if jlens is None:
        jlens = [J] * S
    bf = ml_dtypes.bfloat16
    wT = np.ascontiguousarray(
        Weff.reshape(5, CH, P).transpose(2, 1, 0)).astype(bf)       # (128, CH, 5)
    mb = np.full((P, S, J), np.float32(-30000.0), np.float32)
    for s in range(S):
        js = jlens[s]
        t_idx = np.arange(P)[:, None] * js + np.arange(js)[None, :]
        mb[:, s, 0:js] = np.where(t_idx < int(lens_shard[s]),
                                  np.float32(beff), np.float32(-30000.0))
    m = dict(x=np.ascontiguousarray(x_shard, np.float32))
    m.update(make_consts(S, T, D, L, wT, mb))
    return m


def build_kernel(nc, tc, S, T, D, L, repeats=1, jlens=None):
    from concourse import mybir, bass_isa
    f32, bf16 = mybir.dt.float32, mybir.dt.bfloat16
    AF = mybir.ActivationFunctionType
    OP = mybir.AluOpType
    J = T // P
    CH = D // P
    JG = J // 4   # j-groups of 4

    NF = P + 1 + 2 * L * J + S * J + P
    NB = P + 5 + 2 * P + CH * 5
    x_d = nc.declare_dram_parameter("x", [S, T, D], f32, isOutput=False)
    cf_d = nc.declare_dram_parameter("cf32", [P, NF], f32, isOutput=False)
    cb_d = nc.declare_dram_parameter("cbf16", [P, NB], bf16, isOutput=False)
    out_d = nc.declare_dram_parameter("out", [S, L, D], f32, isOutput=True)

    cpool = tc.alloc_tile_pool(name="consts", bufs=1)
    cb = cpool.tile([P, NB], bf16, tag="cb", name="cb")
    nc.sync.dma_start(cb[:], cb_d[:])
    cf = cpool.tile([P, NF], f32, tag="cf", name="cf")
    nc.sync.dma_start(cf[:], cf_d[:])

    o = 0
    u128x = cf[:, o:o + P]; o += P
    onescol = cf[:, o:o + 1]; o += 1
    eglo = cf[:, o:o + L * J].rearrange("p (l j) -> p l j", j=J); o += L * J
    eghi = cf[:, o:o + L * J].rearrange("p (l j) -> p l j", j=J); o += L * J
    mb = cf[:, o:o + S * J].rearrange("p (s j) -> p s j", j=J); o += S * J
    onesrow = cf[0:1, o:o + P]; o += P
    o = 0
    ident = cb[:, o:o + P]; o += P
    i5 = cb[0:5, o:o + 5]; o += 5
    sup = cb[:, o:o + P]; o += P
    sdn = cb[:, o:o + P]; o += P
    wT = cb[:, o:o + CH * 5].rearrange("p (c k) -> p c k", k=5); o += CH * 5

    xbf_pool = tc.alloc_tile_pool(name="xbf", bufs=1)
    xbfs = [xbf_pool.tile([P, jlens[s], D], bf16, tag=f"xbf{s}", name=f"xbf{s}")
            for s in range(S)]

    xf_pool = tc.alloc_tile_pool(name="xf", bufs=4)
    xt_pool = tc.alloc_tile_pool(name="xt", bufs=2)
    z_pool = tc.alloc_tile_pool(name="zsb", bufs=1)
    zt_pool = tc.alloc_tile_pool(name="ztsb", bufs=1)
    sm_pool = tc.alloc_tile_pool(name="smsb", bufs=2)
    w_pool = tc.alloc_tile_pool(name="wsb", bufs=2)
    o_pool = tc.alloc_tile_pool(name="osb", bufs=2)

    tp_ps = tc.alloc_tile_pool(name="tp_ps", bufs=3, space="PSUM")
    z_ps_pool = tc.alloc_tile_pool(name="z_ps", bufs=1, space="PSUM")
    e_ps_pool = tc.alloc_tile_pool(name="e_ps", bufs=1, space="PSUM")
    s_ps_pool = tc.alloc_tile_pool(name="s_ps", bufs=2, space="PSUM")

    zsbs = [z_pool.tile([5, jlens[s], P], bf16, tag=f"z{s}", name=f"z{s}")
            for s in range(S)]
    ztps = [zt_pool.tile([P, jlens[s] + 4, 5], bf16, tag=f"zt{s}", name=f"zt{s}")
            for s in range(S)]

    def body():
        cv = [0]
        tpc = [0]
        for s in sorted(range(S), key=lambda q: -jlens[q]):
            Js = jlens[s]
            NG = -(-Js // 4)
            xbf = xbfs[s]
            zsb = zsbs[s]
            ztp = ztps[s]
            xsrc = x_d[s][0:P * Js].rearrange("(p j) d -> p j d", j=Js)
            for g in range(NG):
                j0 = 4 * g
                wg = min(4, Js - j0)
                xf = xf_pool.tile([P, 4, D], f32, tag="xf", name="xf")
                nc.sync.dma_start(xf[:, 0:wg, :], xsrc[:, j0:j0 + wg, :])
                conv_out = xbf[:, j0:j0 + wg, :]
                ci = cv[0] % 16; cv[0] += 1
                if ci % 2 == 0:
                    nc.vector.tensor_copy(conv_out, xf[:, 0:wg, :])
                else:
                    nc.scalar.copy(conv_out, xf[:, 0:wg, :])
                z_ps = z_ps_pool.tile([5, 4 * P], f32, tag="zps", name="zps")
                for c in range(CH):
                    tp = tp_ps.tile([P, 4 * P], bf16, tag="tp", name="tp")
                    for q in range(wg):
                        nc.tensor.transpose(
                            tp[:, q * P:(q + 1) * P],
                            xbf[:, j0 + q, c * P:(c + 1) * P],
                            ident,
                        )
                    xt = xt_pool.tile([P, 4 * P], bf16, tag="xt", name="xt")
                    ti = tpc[0] % 16; tpc[0] += 1
                    xt32 = xt[:, 0:wg * P].bitcast(f32)
                    tp32 = tp[:, 0:wg * P].bitcast(f32)
                    if ti < 7:
                        nc.vector.tensor_copy(xt32, tp32)
                    else:
                        nc.scalar.copy(xt32, tp32)
                    nc.tensor.matmul(z_ps[:, 0:wg * P], wT[:, c, :],
                                     xt[:, 0:wg * P],
                                     start=(c == 0), stop=(c == CH - 1))
                nc.vector.tensor_copy(
                    zsb[:, j0:j0 + wg, :],
                    z_ps[:, 0:wg * P].rearrange("k (a p) -> k a p", p=P))
                zt_psl = s_ps_pool.tile([P, 20], f32, tag="smps", name="ztps")
                for q in range(wg):
                    j = j0 + q
                    nc.tensor.matmul(zt_psl[:, q * 5:(q + 1) * 5],
                                     zsb[:, j, :], i5, start=True, stop=True)
                nc.vector.tensor_copy(
                    ztp[:, 2 + j0:2 + j0 + wg, :].rearrange("p a k -> p (a k)"),
                    zt_psl[:, 0:wg * 5])
                if g == 0:
                    fill = s_ps_pool.tile([P, 10], f32, tag="smps", name="fill")
                    nc.tensor.matmul(fill[:], sup,
                                     ztp[:, 2:4, :].rearrange("p a k -> p (a k)"),
                                     start=True, stop=True)
                    nc.vector.tensor_copy(
                        ztp[:, Js + 2:Js + 4, :].rearrange("p a k -> p (a k)"),
                        fill[:])
            fill2 = s_ps_pool.tile([P, 10], f32, tag="smps", name="fill")
            nc.tensor.matmul(fill2[:], sdn,
                             ztp[:, Js:Js + 2, :].rearrange("p a k -> p (a k)"),
                             start=True, stop=True)
            nc.vector.tensor_copy(
                ztp[:, 0:2, :].rearrange("p a k -> p (a k)"), fill2[:])
            # logits[p,j] = sum_k ztp[p, j+k, k]
            lg1 = sm_pool.tile([P, J], f32, tag="lg1", name="lg1")
            lg2 = sm_pool.tile([P, J], f32, tag="lg2", name="lg2")
            lg = sm_pool.tile([P, J], f32, tag="lg", name="lg")
            lg3 = sm_pool.tile([P, J], f32, tag="lg3", name="lg3")
            nc.vector.tensor_add(lg1[:, 0:Js], ztp[:, 2:2 + Js, 2], ztp[:, 3:3 + Js, 3])
            nc.vector.tensor_add(lg2[:, 0:Js], ztp[:, 4:4 + Js, 4], ztp[:, 1:1 + Js, 1])
            nc.vector.scalar_tensor_tensor(
                lg3[:, 0:Js], ztp[:, 0:Js, 0], 0.0, mb[:, s, 0:Js],
                op0=OP.add, op1=OP.add)
            nc.vector.tensor_add(lg1[:, 0:Js], lg1[:, 0:Js], lg2[:, 0:Js])
            nc.vector.tensor_add(lg[:, 0:Js], lg1[:, 0:Js], lg3[:, 0:Js])
            al = sm_pool.tile([P, J], f32, tag="al", name="al")
            tot = sm_pool.tile([P, 1], f32, tag="tot", name="tot")
            nc.scalar.activation(al[:, 0:Js], lg[:, 0:Js], AF.Sigmoid,
                                 accum_out=tot[:])
            A0 = sm_pool.tile([P, J], f32, tag="A0", name="A0")
            nc.vector.tensor_tensor_scan(A0[:, 0:Js], al[:, 0:Js], al[:, 0:Js],
                                         0.0, op0=OP.add, op1=OP.bypass)
            offs = s_ps_pool.tile([P, 1], f32, tag="smps", name="offs")
            nc.tensor.matmul(offs[:], u128x, tot[:], start=True, stop=True)
            total = s_ps_pool.tile([1, 1], f32, tag="smps", name="total")
            nc.tensor.matmul(total[:], onescol, tot[:], start=True, stop=True)
            rec = sm_pool.tile([1, 1], f32, tag="rec", name="rec")
            nc.vector.reciprocal(rec[:], total[:])
            sbc = s_ps_pool.tile([P, 1], f32, tag="smps", name="sbc")
            nc.tensor.matmul(sbc[:], onesrow, rec[:], start=True, stop=True)
            sbv = sm_pool.tile([P, 1], f32, tag="sbv", name="sbv")
            nc.vector.tensor_copy(sbv[:], sbc[:])
            A1 = sm_pool.tile([P, J], f32, tag="A1", name="A1")
            nc.vector.tensor_scalar_add(A1[:, 0:Js], A0[:, 0:Js], offs[:, 0:1])
            An = sm_pool.tile([P, J], f32, tag="An", name="An")
            nc.vector.tensor_mul(An[:, 0:Js], A1[:, 0:Js],
                                 sbv[:].broadcast_to([P, Js]))
            aln = sm_pool.tile([P, J], f32, tag="aln", name="aln")
            nc.vector.tensor_mul(aln[:, 0:Js], al[:, 0:Js],
                                 sbv[:].broadcast_to([P, Js]))
            Ap = sm_pool.tile([P, J], f32, tag="Ap", name="Ap")
            nc.vector.tensor_sub(Ap[:, 0:Js], An[:, 0:Js], aln[:, 0:Js])
            lo = w_pool.tile([P, L, J], f32, tag="lo", name="lo")
            nc.vector.tensor_max(
                lo[:, :, 0:Js],
                Ap[:, 0:Js].rearrange("p (o j) -> p o j", o=1).broadcast_to([P, L, Js]),
                eglo[:, :, 0:Js])
            hi = w_pool.tile([P, L, J], f32, tag="hi", name="hi")
            nc.vector.tensor_tensor(
                hi[:, :, 0:Js],
                An[:, 0:Js].rearrange("p (o j) -> p o j", o=1).broadcast_to([P, L, Js]),
                eghi[:, :, 0:Js], op=OP.min)
            wd = w_pool.tile([P, L, J], f32, tag="wd", name="wd")
            nc.vector.tensor_sub(wd[:, :, 0:Js], hi[:, :, 0:Js], lo[:, :, 0:Js])
            wbf = w_pool.tile([P, L, J], bf16, tag="wbf", name="wbf")
            nc.vector.tensor_scalar_max(wbf[:, :, 0:Js], wd[:, :, 0:Js], 0.0)
            e_ps = e_ps_pool.tile([L, 2, 512], f32, tag="eps", name="eps")
            for j in range(Js):
                for h in range(2):
                    nc.tensor.matmul(e_ps[:, h, 0:D // 2], wbf[:, :, j],
                                     xbf[:, j, h * (D // 2):(h + 1) * (D // 2)],
                                     start=(j == 0), stop=(j == Js - 1))
            osb = o_pool.tile([L, D], f32, tag="osb", name="osb")
            nc.vector.tensor_copy(osb[:, 0:D // 2], e_ps[:, 0, 0:D // 2])
            nc.vector.tensor_copy(osb[:, D // 2:D], e_ps[:, 1, 0:D // 2])
            nc.sync.dma_start(out_d[s], osb[:])

    if repeats == 1:
        body()
    else:
        with tc.For_i(0, repeats, 1):
            body()
    for pool in [s_ps_pool, e_ps_pool, z_ps_pool, tp_ps, o_pool, w_pool,
                 sm_pool, zt_pool, z_pool, xt_pool, xf_pool, xbf_pool, cpool]:
        pool.release()
    return nc


# ---------------------------------------------------------------------------
# Runner (persistent jitted SPMD dispatch via PJRT under axon)
# ---------------------------------------------------------------------------

_CACHE = {}


def _get_runner(repeats=1, jlens=None):
    key = ("runner", repeats, tuple(jlens) if jlens else None)
    if key in _CACHE:
        return _CACHE[key]
    import concourse.tile as tile
    from concourse import bacc

    nc = bacc.Bacc()
    with tile.TileContext(nc) as tc:
        build_kernel(nc, tc, S_PER_CORE, T_FULL, D_FULL, L_OUT,
                     repeats=repeats, jlens=jlens)
    nc.compile()
    runner = _SpmdRunner(nc, N_CORES)
    _CACHE[key] = runner
    return runner


def plan_shards(encoder_lens):
    """Sort seqs by length; slot i of core c gets sorted[i*N_CORES + c].
    Returns (perm, jlens): jlens[i] = per-partition j-extent (t = p*Js + j),
    sized so 128*Js covers maxlen+2 (conv halo)."""
    order = np.argsort(np.asarray(encoder_lens), kind="stable")
    perm = np.empty(BSZ, np.int64)
    jlens = []
    J = T_FULL // P
    for i in range(S_PER_CORE):
        grp = order[i * N_CORES:(i + 1) * N_CORES]
        maxlen = int(np.asarray(encoder_lens)[grp].max())
        js = min(J, -(-(maxlen + 2) // P))
        jlens.append(js)
        for c in range(N_CORES):
            perm[c * S_PER_CORE + i] = grp[c]
    return perm, jlens


class _SpmdRunner:
    def __init__(self, nc, n_cores):
        import jax
        import concourse.mybir as mybir
        from concourse.bass2jax import (_bass_exec_p, partition_id_tensor,
                                        install_neuronx_cc_hook)
        from jax.sharding import Mesh, PartitionSpec
        from jax.experimental.shard_map import shard_map

        install_neuronx_cc_hook()
        self.jax = jax
        self.nc = nc
        self.n_cores = n_cores
        partition_name = (nc.partition_id_tensor.name
                          if nc.partition_id_tensor else None)
        in_names, out_names, out_avals, zero_outs = [], [], [], []
        for alloc in nc.m.functions[0].allocations:
            if not isinstance(alloc, mybir.MemoryLocationSet):
                continue
            name = alloc.memorylocations[0].name
            if alloc.kind == "ExternalInput":
                if name != partition_name:
                    in_names.append(name)
            elif alloc.kind == "ExternalOutput":
                out_names.append(name)
                shape = tuple(alloc.tensor_shape)
                dtype = mybir.dt.np(alloc.dtype)
                out_avals.append(jax.core.ShapedArray(shape, dtype))
                zero_outs.append(np.zeros(shape, dtype))
        self.in_names, self.out_names = in_names, out_names
        self.out_avals, self.zero_outs = out_avals, zero_outs
        n_params = len(in_names)
        self.n_params = n_params
        all_in_names = list(in_names) + list(out_names)
        if partition_name is not None:
            all_in_names.append(partition_name)

        def _body(*args):
            operands = list(args)
            if partition_name is not None:
                operands.append(partition_id_tensor())
            outs = _bass_exec_p.bind(
                *operands,
                out_avals=tuple(out_avals),
                in_names=tuple(all_in_names),
                out_names=tuple(out_names),
                lowering_input_output_aliases=(),
                sim_require_finite=True,
                sim_require_nnan=True,
                nc=nc,
            )
            return tuple(outs)

        devices = jax.devices()[:n_cores]
        self.mesh = Mesh(np.asarray(devices), ("core",))
        n_outs = len(out_names)
        in_specs = (PartitionSpec("core"),) * (n_params + n_outs)
        out_specs = (PartitionSpec("core"),) * n_outs
        self.fn = jax.jit(
            shard_map(_body, mesh=self.mesh, in_specs=in_specs,
                      out_specs=out_specs, check_rep=False),
            keep_unused=True,
        )
        self._psharding = jax.sharding.NamedSharding(self.mesh,
                                                     PartitionSpec("core"))

    def device_inputs(self, in_maps):
        jax = self.jax
        per_core = [[np.asarray(m[n]) for n in self.in_names] for m in in_maps]
        concat_in = [
            np.concatenate([per_core[c][i] for c in range(self.n_cores)], axis=0)
            for i in range(self.n_params)
        ]
        concat_zeros = [
            np.zeros((self.n_cores * z.shape[0], *z.shape[1:]), z.dtype)
            for z in self.zero_outs
        ]
        return [jax.device_put(a, self._psharding)
                for a in concat_in + concat_zeros]

    def run(self, in_maps):
        jax = self.jax
        dev_in = self.device_inputs(in_maps)
        outs = self.fn(*dev_in)
        jax.block_until_ready(outs)
        return [
            {n: np.asarray(outs[i]).reshape(self.n_cores,
                                            *self.out_avals[i].shape)[c]
             for i, n in enumerate(self.out_names)}
            for c in range(self.n_cores)
        ]


def _make_in_maps(encoder_outputs, encoder_lens, conv_w, conv_b, lin_w, lin_b,
                  perm=None, jlens=None):
    Weff, beff = fold_weights(conv_w, conv_b, lin_w, lin_b)
    x = np.asarray(encoder_outputs, np.float32)
    lens = np.asarray(encoder_lens)
    if perm is not None:
        x = x[perm]
        lens = lens[perm]
    in_maps = []
    for c in range(N_CORES):
        sl = slice(c * S_PER_CORE, (c + 1) * S_PER_CORE)
        in_maps.append(make_core_inputs(
            x[sl], lens[sl], Weff, beff, S_PER_CORE, T_FULL, D_FULL, L_OUT,
            jlens=jlens))
    return in_maps


def kernel(encoder_outputs, encoder_lens, conv_w, conv_b, lin_w, lin_b):
    perm, jlens = plan_shards(encoder_lens)
    runner = _get_runner(repeats=1, jlens=jlens)
    in_maps = _make_in_maps(encoder_outputs, encoder_lens,
                            conv_w, conv_b, lin_w, lin_b, perm=perm,
                            jlens=jlens)
    res = runner.run(in_maps)
    permuted = np.concatenate([res[c]["out"] for c in range(N_CORES)], axis=0)
    out = np.empty_like(permuted)
    out[perm] = permuted
    return out



# revision 14
# speedup vs baseline: 1.7064x; 1.7064x over previous
"""CIF (Continuous Integrate-and-Fire) forced-alignment kernel for 8 TRN2 NeuronCores.

Contract: kernel(**inputs) takes the FULL inputs from setup_inputs() and returns
the FULL (32, 8, 768) f32 output. Batch is sharded 4 seqs/core (pure data
parallel); each core runs an identical Bass/Tile program.

Math: Conv1d(D,D,5,SAME) + Linear(D,1) collapse into an effective (5,D) filter
(both are linear); sigmoid -> mask -> normalize to sum L -> cumsum -> bucket
overlap weights w (B,L,T) -> out = w @ x.

v2: x is uploaded as bf16 (halves HBM traffic; the device math was already
bf16). Per token-column j: PE-transpose all 6 d-chunks into one PSUM bank,
one copy to SBUF, then 6 tiny accumulating matmuls (x-block stationary, W
moving) produce z token-major [t,5] directly in the ztp layout -- no narrow
5-partition copies and no per-j transpose-back matmuls.
"""
import sys

if "/opt/trn_rl_repo" not in sys.path:
    sys.path.insert(0, "/opt/trn_rl_repo")

import numpy as np
import ml_dtypes

P = 128
BSZ, T_FULL, D_FULL = 32, 2048, 768
L_OUT = 8
N_CORES = 8
S_PER_CORE = BSZ // N_CORES


def fold_weights(conv_w, conv_b, lin_w, lin_b):
    Weff = np.einsum("o,oik->ki", lin_w[0].astype(np.float64),
                     conv_w.astype(np.float64)).astype(np.float32)  # (5, D)
    beff = float(lin_w[0].astype(np.float64) @ conv_b.astype(np.float64) + lin_b[0])
    return Weff, beff


def make_consts(S, T, D, L, wT, mb):
    """Pack all constants into one f32 blob and one bf16 blob (2 DMAs)."""
    J = T // P
    CH = D // P
    bf = ml_dtypes.bfloat16
    u128x = np.triu(np.ones((P, P), np.float32), 1)         # [p,m]=1 iff p<m
    onescol = np.ones((P, 1), np.float32)
    l = np.arange(L, dtype=np.float32)
    eglo = np.broadcast_to(l[None, :, None], (P, L, J)).astype(np.float32)
    eghi = eglo + 1.0
    onesrow_slot = np.zeros((P, P), np.float32)
    onesrow_slot[0, :] = np.float32(L)
    sup = np.zeros((P, P), np.float32)                      # out[m]=in[m+1]
    sup[np.arange(1, P), np.arange(P - 1)] = 1
    sdn = np.zeros((P, P), np.float32)                      # out[m]=in[m-1]
    sdn[np.arange(P - 1), np.arange(1, P)] = 1
    cf32 = np.concatenate([
        u128x, onescol, eglo.reshape(P, L * J), eghi.reshape(P, L * J),
        mb.reshape(P, -1), onesrow_slot, sup, sdn], axis=1)
    ident = np.eye(P, dtype=bf)
    cbf16 = np.concatenate([ident, wT.reshape(P, CH * 5)], axis=1)
    return dict(cf32=np.ascontiguousarray(cf32),
                cbf16=np.ascontiguousarray(cbf16))


def make_core_inputs(x_shard, lens_shard, Weff, beff, S, T, D, L, jlens=None):
    J = T // P
    CH = D // P
    if jlens is None:
        jlens = [J] * S
    bf = ml_dtypes.bfloat16
    wT = np.ascontiguousarray(
        Weff.reshape(5, CH, P).transpose(2, 1, 0)).astype(bf)       # (128, CH, 5)
    mb = np.full((P, S, J), np.float32(-30000.0), np.float32)
    for s in range(S):
        js = jlens[s]
        t_idx = np.arange(P)[:, None] * js + np.arange(js)[None, :]
        mb[:, s, 0:js] = np.where(t_idx < int(lens_shard[s]),
                                  np.float32(beff), np.float32(-30000.0))
    m = dict(x=np.ascontiguousarray(np.asarray(x_shard).astype(bf)))
    m.update(make_consts(S, T, D, L, wT, mb))
    return m


def build_kernel(nc, tc, S, T, D, L, repeats=1, jlens=None):
    from concourse import mybir, bass_isa
    f32, bf16 = mybir.dt.float32, mybir.dt.bfloat16
    AF = mybir.ActivationFunctionType
    OP = mybir.AluOpType
    J = T // P
    CH = D // P
    if jlens is None:
        jlens = [J] * S

    NF = P + 1 + 2 * L * J + S * J + 3 * P
    NB = P + CH * 5
    x_d = nc.declare_dram_parameter("x", [S, T, D], bf16, isOutput=False)
    cf_d = nc.declare_dram_parameter("cf32", [P, NF], f32, isOutput=False)
    cb_d = nc.declare_dram_parameter("cbf16", [P, NB], bf16, isOutput=False)
    out_d = nc.declare_dram_parameter("out", [S, L, D], f32, isOutput=True)

    cpool = tc.alloc_tile_pool(name="consts", bufs=1)
    cb = cpool.tile([P, NB], bf16, tag="cb", name="cb")
    nc.sync.dma_start(cb[:], cb_d[:])
    cf = cpool.tile([P, NF], f32, tag="cf", name="cf")
    nc.sync.dma_start(cf[:], cf_d[:])

    o = 0
    u128x = cf[:, o:o + P]; o += P
    onescol = cf[:, o:o + 1]; o += 1
    eglo = cf[:, o:o + L * J].rearrange("p (l j) -> p l j", j=J); o += L * J
    eghi = cf[:, o:o + L * J].rearrange("p (l j) -> p l j", j=J); o += L * J
    mb = cf[:, o:o + S * J].rearrange("p (s j) -> p s j", j=J); o += S * J
    onesrow = cf[0:1, o:o + P]; o += P
    sup = cf[:, o:o + P]; o += P
    sdn = cf[:, o:o + P]; o += P
    o = 0
    ident = cb[:, o:o + P]; o += P
    wT = cb[:, o:o + CH * 5].rearrange("p (c k) -> p c k", k=5); o += CH * 5

    xbf_pool = tc.alloc_tile_pool(name="xbf", bufs=1)
    xbfs = [xbf_pool.tile([P, jlens[s], D], bf16, tag=f"xbf{s}", name=f"xbf{s}")
            for s in range(S)]

    xt_pool = tc.alloc_tile_pool(name="xt", bufs=6)
    sm_pool = tc.alloc_tile_pool(name="smsb", bufs=2)
    w_pool = tc.alloc_tile_pool(name="wsb", bufs=2)
    o_pool = tc.alloc_tile_pool(name="osb", bufs=2)

    tp_ps = tc.alloc_tile_pool(name="tp_ps", bufs=3, space="PSUM")
    zt_ps_pool = tc.alloc_tile_pool(name="zt_ps", bufs=1, space="PSUM")
    e_ps_pool = tc.alloc_tile_pool(name="e_ps", bufs=1, space="PSUM")
    s_ps_pool = tc.alloc_tile_pool(name="s_ps", bufs=2, space="PSUM")

    zt_pool = tc.alloc_tile_pool(name="ztsb", bufs=1)
    ztps = [zt_pool.tile([P, jlens[s] + 4, 5], f32, tag=f"zt{s}", name=f"zt{s}")
            for s in range(S)]

    def body():
        cv = [0]
        order = sorted(range(S), key=lambda q: -jlens[q])
        wbfs = {}

        def dma_x(s):
            Js = jlens[s]
            xsrc = x_d[s][0:P * Js].rearrange("(p j) d -> p j d", j=Js)
            for g in range(-(-Js // 4)):
                j0 = 4 * g
                wg = min(4, Js - j0)
                nc.sync.dma_start(xbfs[s][:, j0:j0 + wg, :],
                                  xsrc[:, j0:j0 + wg, :])

        def a_z(s):
            """Transposes + PSUM->SBUF copy + token-major z matmuls + sup halo.

            The z matmul for token-column j is issued two j's behind its
            transposes so the PE never waits on the PSUM->SBUF copy (in-order
            engine queues)."""
            Js = jlens[s]
            NG = -(-Js // 4)
            xbf = xbfs[s]
            ztp = ztps[s]
            SKEW = 3
            xts = {}
            zt_tiles = {}

            def do_zmm(j):
                g = j // 4
                q = j % 4
                if q == 0:
                    zt_tiles[g] = zt_ps_pool.tile([P, 4, 5], f32, tag="ztps",
                                                  name="ztps")
                zt_psl = zt_tiles[g]
                xt = xts.pop(j)
                for c in range(CH):
                    nc.tensor.matmul(zt_psl[:, q, :],
                                     xt[:, c * P:(c + 1) * P],
                                     wT[:, c, :],
                                     start=(c == 0), stop=(c == CH - 1))
                j0 = 4 * g
                wg = min(4, Js - j0)
                if q == wg - 1:
                    nc.vector.tensor_copy(
                        ztp[:, 2 + j0:2 + j0 + wg, :].rearrange("p a k -> p (a k)"),
                        zt_psl[:, 0:wg, :].rearrange("p a k -> p (a k)"))
                    if g == 0:
                        fill = s_ps_pool.tile([P, 10], f32, tag="smps",
                                              name="fill")
                        nc.tensor.matmul(
                            fill[:], sup,
                            ztp[:, 2:4, :].rearrange("p a k -> p (a k)"),
                            start=True, stop=True)
                        nc.vector.tensor_copy(
                            ztp[:, Js + 2:Js + 4, :].rearrange("p a k -> p (a k)"),
                            fill[:])

            for j in range(Js):
                tp = tp_ps.tile([P, D], bf16, tag="tp", name="tp")
                for c in range(CH):
                    nc.tensor.transpose(
                        tp[:, c * P:(c + 1) * P],
                        xbf[:, j, c * P:(c + 1) * P],
                        ident,
                    )
                xt = xt_pool.tile([P, D], bf16, tag="xt", name="xt")
                ci = cv[0] % 2; cv[0] += 1
                xt32 = xt[:].bitcast(f32)
                tp32 = tp[:].bitcast(f32)
                if ci == 0:
                    nc.vector.tensor_copy(xt32, tp32)
                else:
                    nc.scalar.copy(xt32, tp32)
                xts[j] = xt
                if j >= SKEW:
                    do_zmm(j - SKEW)
            for j in range(max(0, Js - SKEW), Js):
                do_zmm(j)

        def tail(s):
            """sdn halo + logits + sigmoid + cumsum + normalize + w weights."""
            Js = jlens[s]
            ztp = ztps[s]
            fill2 = s_ps_pool.tile([P, 10], f32, tag="smps", name="fill")
            nc.tensor.matmul(fill2[:], sdn,
                             ztp[:, Js:Js + 2, :].rearrange("p a k -> p (a k)"),
                             start=True, stop=True)
            nc.vector.tensor_copy(
                ztp[:, 0:2, :].rearrange("p a k -> p (a k)"), fill2[:])
            # logits[p,j] = sum_k ztp[p, j+k, k]
            lg1 = sm_pool.tile([P, J], f32, tag="lg1", name="lg1")
            lg2 = sm_pool.tile([P, J], f32, tag="lg2", name="lg2")
            lg = sm_pool.tile([P, J], f32, tag="lg", name="lg")
            lg3 = sm_pool.tile([P, J], f32, tag="lg3", name="lg3")
            nc.gpsimd.tensor_add(lg1[:, 0:Js], ztp[:, 2:2 + Js, 2], ztp[:, 3:3 + Js, 3])
            nc.vector.tensor_add(lg2[:, 0:Js], ztp[:, 4:4 + Js, 4], ztp[:, 1:1 + Js, 1])
            nc.vector.scalar_tensor_tensor(
                lg3[:, 0:Js], ztp[:, 0:Js, 0], 0.0, mb[:, s, 0:Js],
                op0=OP.add, op1=OP.add)
            nc.gpsimd.tensor_add(lg1[:, 0:Js], lg1[:, 0:Js], lg2[:, 0:Js])
            nc.gpsimd.tensor_add(lg[:, 0:Js], lg1[:, 0:Js], lg3[:, 0:Js])
            al = sm_pool.tile([P, J], f32, tag="al", name="al")
            tot = sm_pool.tile([P, 1], f32, tag="tot", name="tot")
            nc.scalar.activation(al[:, 0:Js], lg[:, 0:Js], AF.Sigmoid,
                                 accum_out=tot[:])
            A0 = sm_pool.tile([P, J], f32, tag="A0", name="A0")
            nc.vector.tensor_tensor_scan(A0[:, 0:Js], al[:, 0:Js], al[:, 0:Js],
                                         0.0, op0=OP.add, op1=OP.bypass)
            offs = s_ps_pool.tile([P, 1], f32, tag="smps", name="offs")
            nc.tensor.matmul(offs[:], u128x, tot[:], start=True, stop=True)
            total = s_ps_pool.tile([1, 1], f32, tag="smps", name="total")
            nc.tensor.matmul(total[:], onescol, tot[:], start=True, stop=True)
            rec = sm_pool.tile([1, 1], f32, tag="rec", name="rec")
            nc.vector.reciprocal(rec[:], total[:])
            sbc = s_ps_pool.tile([P, 1], f32, tag="smps", name="sbc")
            nc.tensor.matmul(sbc[:], onesrow, rec[:], start=True, stop=True)
            sbv = sm_pool.tile([P, 1], f32, tag="sbv", name="sbv")
            nc.vector.tensor_copy(sbv[:], sbc[:])
            A1 = sm_pool.tile([P, J], f32, tag="A1", name="A1")
            nc.vector.tensor_scalar_add(A1[:, 0:Js], A0[:, 0:Js], offs[:, 0:1])
            An = sm_pool.tile([P, J], f32, tag="An", name="An")
            nc.vector.tensor_mul(An[:, 0:Js], A1[:, 0:Js],
                                 sbv[:].broadcast_to([P, Js]))
            aln = sm_pool.tile([P, J], f32, tag="aln", name="aln")
            nc.vector.tensor_mul(aln[:, 0:Js], al[:, 0:Js],
                                 sbv[:].broadcast_to([P, Js]))
            Ap = sm_pool.tile([P, J], f32, tag="Ap", name="Ap")
            nc.gpsimd.tensor_sub(Ap[:, 0:Js], An[:, 0:Js], aln[:, 0:Js])
            lo = w_pool.tile([P, L, J], f32, tag="lo", name="lo")
            nc.vector.tensor_max(
                lo[:, :, 0:Js],
                Ap[:, 0:Js].rearrange("p (o j) -> p o j", o=1).broadcast_to([P, L, Js]),
                eglo[:, :, 0:Js])
            hi = w_pool.tile([P, L, J], f32, tag="hi", name="hi")
            nc.vector.tensor_tensor(
                hi[:, :, 0:Js],
                An[:, 0:Js].rearrange("p (o j) -> p o j", o=1).broadcast_to([P, L, Js]),
                eghi[:, :, 0:Js], op=OP.min)
            wd = w_pool.tile([P, L, J], f32, tag="wd", name="wd")
            nc.gpsimd.tensor_sub(wd[:, :, 0:Js], hi[:, :, 0:Js], lo[:, :, 0:Js])
            wbf = w_pool.tile([P, L, J], bf16, tag="wbf", name="wbf")
            nc.vector.tensor_scalar_max(wbf[:, :, 0:Js], wd[:, :, 0:Js], 0.0)
            wbfs[s] = wbf

        def b_out(s):
            """Einsum out = w @ x, copy out, store."""
            Js = jlens[s]
            xbf = xbfs[s]
            wbf = wbfs[s]
            e_ps = e_ps_pool.tile([L, 2, 512], f32, tag="eps", name="eps")
            for j in range(Js):
                for h in range(2):
                    nc.tensor.matmul(e_ps[:, h, 0:D // 2], wbf[:, :, j],
                                     xbf[:, j, h * (D // 2):(h + 1) * (D // 2)],
                                     start=(j == 0), stop=(j == Js - 1))
            osb = o_pool.tile([L, D], f32, tag="osb", name="osb")
            nc.scalar.copy(osb[:, 0:D // 2], e_ps[:, 0, 0:D // 2])
            nc.scalar.copy(osb[:, D // 2:D], e_ps[:, 1, 0:D // 2])
            nc.sync.dma_start(out_d[s], osb[:])

        # Software pipeline: z-phase of seq i+1 issues before the latency-bound
        # tail of seq i; the einsum of seq i lands two slots later so its w is
        # ready when the PE queue reaches it.
        for s in order:
            dma_x(s)
        slots = []
        for i, s in enumerate(order):
            slots.append(("a", s))
        prog = []
        prog.append(("a", order[0]))
        prog.append(("a", order[1]))
        prog.append(("t", order[0]))
        prog.append(("a", order[2]))
        prog.append(("t", order[1]))
        prog.append(("b", order[0]))
        prog.append(("a", order[3]))
        prog.append(("t", order[2]))
        prog.append(("b", order[1]))
        prog.append(("t", order[3]))
        prog.append(("b", order[2]))
        prog.append(("b", order[3]))
        for kind, s in prog:
            if kind == "a":
                a_z(s)
            elif kind == "t":
                tail(s)
            else:
                b_out(s)

    if repeats == 1:
        body()
    else:
        with tc.For_i(0, repeats, 1):
            body()
    for pool in [zt_pool, s_ps_pool, e_ps_pool, zt_ps_pool, tp_ps, o_pool,
                 w_pool, sm_pool, xt_pool, xbf_pool, cpool]:
        pool.release()
    return nc


# ---------------------------------------------------------------------------
# Runner (persistent jitted SPMD dispatch via PJRT under axon)
# ---------------------------------------------------------------------------

_CACHE = {}


def _get_runner(repeats=1, jlens=None):
    key = ("runner", repeats, tuple(jlens) if jlens else None)
    if key in _CACHE:
        return _CACHE[key]
    import concourse.tile as tile
    from concourse import bacc

    nc = bacc.Bacc()
    with tile.TileContext(nc) as tc:
        build_kernel(nc, tc, S_PER_CORE, T_FULL, D_FULL, L_OUT,
                     repeats=repeats, jlens=jlens)
    nc.compile()
    runner = _SpmdRunner(nc, N_CORES)
    _CACHE[key] = runner
    return runner


def plan_shards(encoder_lens):
    """Sort seqs by length; slot i of core c gets sorted[i*N_CORES + c].
    Returns (perm, jlens): jlens[i] = per-partition j-extent (t = p*Js + j),
    sized so 128*Js covers maxlen+2 (conv halo)."""
    order = np.argsort(np.asarray(encoder_lens), kind="stable")
    perm = np.empty(BSZ, np.int64)
    jlens = []
    J = T_FULL // P
    for i in range(S_PER_CORE):
        grp = order[i * N_CORES:(i + 1) * N_CORES]
        maxlen = int(np.asarray(encoder_lens)[grp].max())
        js = min(J, -(-(maxlen + 2) // P))
        jlens.append(js)
        for c in range(N_CORES):
            perm[c * S_PER_CORE + i] = grp[c]
    return perm, jlens


class _SpmdRunner:
    def __init__(self, nc, n_cores):
        import jax
        import concourse.mybir as mybir
        from concourse.bass2jax import (_bass_exec_p, partition_id_tensor,
                                        install_neuronx_cc_hook)
        from jax.sharding import Mesh, PartitionSpec
        from jax.experimental.shard_map import shard_map

        install_neuronx_cc_hook()
        self.jax = jax
        self.nc = nc
        self.n_cores = n_cores
        partition_name = (nc.partition_id_tensor.name
                          if nc.partition_id_tensor else None)
        in_names, out_names, out_avals, zero_outs = [], [], [], []
        for alloc in nc.m.functions[0].allocations:
            if not isinstance(alloc, mybir.MemoryLocationSet):
                continue
            name = alloc.memorylocations[0].name
            if alloc.kind == "ExternalInput":
                if name != partition_name:
                    in_names.append(name)
            elif alloc.kind == "ExternalOutput":
                out_names.append(name)
                shape = tuple(alloc.tensor_shape)
                dtype = mybir.dt.np(alloc.dtype)
                out_avals.append(jax.core.ShapedArray(shape, dtype))
                zero_outs.append(np.zeros(shape, dtype))
        self.in_names, self.out_names = in_names, out_names
        self.out_avals, self.zero_outs = out_avals, zero_outs
        n_params = len(in_names)
        self.n_params = n_params
        all_in_names = list(in_names) + list(out_names)
        if partition_name is not None:
            all_in_names.append(partition_name)

        def _body(*args):
            operands = list(args)
            if partition_name is not None:
                operands.append(partition_id_tensor())
            outs = _bass_exec_p.bind(
                *operands,
                out_avals=tuple(out_avals),
                in_names=tuple(all_in_names),
                out_names=tuple(out_names),
                lowering_input_output_aliases=(),
                sim_require_finite=True,
                sim_require_nnan=True,
                nc=nc,
            )
            return tuple(outs)

        devices = jax.devices()[:n_cores]
        self.mesh = Mesh(np.asarray(devices), ("core",))
        n_outs = len(out_names)
        in_specs = (PartitionSpec("core"),) * (n_params + n_outs)
        out_specs = (PartitionSpec("core"),) * n_outs
        self.fn = jax.jit(
            shard_map(_body, mesh=self.mesh, in_specs=in_specs,
                      out_specs=out_specs, check_rep=False),
            keep_unused=True,
        )
        self._psharding = jax.sharding.NamedSharding(self.mesh,
                                                     PartitionSpec("core"))

    def device_inputs(self, in_maps):
        jax = self.jax
        per_core = [[np.asarray(m[n]) for n in self.in_names] for m in in_maps]
        concat_in = [
            np.concatenate([per_core[c][i] for c in range(self.n_cores)], axis=0)
            for i in range(self.n_params)
        ]
        concat_zeros = [
            np.zeros((self.n_cores * z.shape[0], *z.shape[1:]), z.dtype)
            for z in self.zero_outs
        ]
        return [jax.device_put(a, self._psharding)
                for a in concat_in + concat_zeros]

    def run(self, in_maps):
        jax = self.jax
        dev_in = self.device_inputs(in_maps)
        outs = self.fn(*dev_in)
        jax.block_until_ready(outs)
        return [
            {n: np.asarray(outs[i]).reshape(self.n_cores,
                                            *self.out_avals[i].shape)[c]
             for i, n in enumerate(self.out_names)}
            for c in range(self.n_cores)
        ]


def _make_in_maps(encoder_outputs, encoder_lens, conv_w, conv_b, lin_w, lin_b,
                  perm=None, jlens=None):
    Weff, beff = fold_weights(conv_w, conv_b, lin_w, lin_b)
    x = np.asarray(encoder_outputs, np.float32)
    lens = np.asarray(encoder_lens)
    if perm is not None:
        x = x[perm]
        lens = lens[perm]
    in_maps = []
    for c in range(N_CORES):
        sl = slice(c * S_PER_CORE, (c + 1) * S_PER_CORE)
        in_maps.append(make_core_inputs(
            x[sl], lens[sl], Weff, beff, S_PER_CORE, T_FULL, D_FULL, L_OUT,
            jlens=jlens))
    return in_maps


def kernel(encoder_outputs, encoder_lens, conv_w, conv_b, lin_w, lin_b):
    perm, jlens = plan_shards(encoder_lens)
    runner = _get_runner(repeats=1, jlens=jlens)
    in_maps = _make_in_maps(encoder_outputs, encoder_lens,
                            conv_w, conv_b, lin_w, lin_b, perm=perm,
                            jlens=jlens)
    res = runner.run(in_maps)
    permuted = np.concatenate([res[c]["out"] for c in range(N_CORES)], axis=0)
    out = np.empty_like(permuted)
    out[perm] = permuted
    return out
